# revision 42
# baseline (speedup 1.0000x reference)
"""BiAttention similarity kernel for Trainium2, 8-core data-parallel over batch.

Computes, per batch b:
    s0 = c @ c_weight                  # [L, 1]
    s1 = (c @ q_weight)^T              # [1, L]
    s2 = (c * cq_weight) @ q^T         # [L, L]
    s  = s0 + s1 + s2 + bias           # [L, L]

Shapes (hardcoded): B=8, L=2048, D=256, fp32 in/out (device math fp16/fp32,
device output fp16, upcast to fp32 on host).

Distribution: data-parallel over batch, one batch per core. Host hands each
core its shard d-major (transposed) fp16 plus pre-packed per-partition weight
tiles; device output is fp16 (halves the dominant HBM write).

Device plan per core:
  - warmup matmuls on constant data during the input-load window so the PE's
    HAM clock gate reaches 2.4 GHz before real work arrives
  - S1B [128, L]: s1[j]+bias broadcast across partitions, computed on the PE
    as qw_bcast^T @ cT (+ ones^T @ bias_row), evacuated to SBUF fp16
  - s0 column accumulated in one PSUM bank via N=1 matmuls that reuse the
    main loop's stationary cT chunks, bounced [128,1] per chunk to SBUF
  - main GEMM: 16 row chunks; PSUM tiles are [128,1024] fp32 (two banks) so
    each evacuation instruction covers two banks' worth of output
  - evacuation fuses the rank-1 adds, split across three engines:
      a tiles: DVE scalar_tensor_tensor  out = (psum + s0col) + S1B
      b tiles: ScalarE activation (psum + s0col -> fp16 tmp), then
               DVE tensor_tensor (2x fp16 mode) or GpSimd adds S1B
  - output: one 512 KiB DMA per row chunk on the Sync HWDGE ring
"""

import numpy as np
from contextlib import ExitStack

import concourse.bass as bass
import concourse.tile as tile
from concourse import bacc, mybir
from concourse.bass_utils import run_bass_kernel_spmd

F32 = mybir.dt.float32
F16 = mybir.dt.float16

B = 8
L = 2048
D = 256
NK = D // 128          # 2 contraction chunks of 128
NI = L // 128          # 16 row chunks
MAIN_N = 512           # one matmul output <= one PSUM bank
BIG_N = 1024           # evacuation tile: two PSUM banks
N_WARM = 10            # PE warmup matmuls during input load
GPS_CHUNKS = {0, 1, 2, 4, 5, 6, 8, 9, 10, 12, 13}  # b-tile partner GpSimd

TRACE = False
LAST_RESULTS = None

_NC_CACHE = None


def build_body(ctx: ExitStack, tc: tile.TileContext, aps: dict):
    nc = tc.nc
    ct_d, qt_d, wflat_d, s_d = (
        aps["ct"], aps["qt"], aps["wflat"], aps["s"],
    )
    Copy = mybir.ActivationFunctionType.Copy
    Ident = mybir.ActivationFunctionType.Identity
    ADD = mybir.AluOpType.add

    consts = ctx.enter_context(tc.tile_pool(name="consts", bufs=1))
    # PSUM budget (8 banks): pa ring 3 x [128,512] = 3 banks, pb ring
    # 2 x [128,1024] = 4 banks (2-chunk depth on the tight edge), s0acc 1
    psum_pa = ctx.enter_context(tc.tile_pool(name="psum_pa", bufs=3,
                                             space="PSUM"))
    psum_pb = ctx.enter_context(tc.tile_pool(name="psum_pb", bufs=2,
                                             space="PSUM"))
    psum_s0 = ctx.enter_context(tc.tile_pool(name="psum_s0", bufs=1,
                                             space="PSUM"))
    outp = ctx.enter_context(tc.tile_pool(name="outp", bufs=4))
    tmpp = ctx.enter_context(tc.tile_pool(name="tmpp", bufs=4))

    # ---- packed constants: one fast HWDGE load each -----------------------
    # w16[p] = [cw[p], cw[128+p]] fp16; w32[p] = [cqw k0, cqw k1, qw k0,
    # qw k1, bias] fp32 (bias only meaningful on partition 0).
    # cT[0] is issued first on the sync ring: it gates the S1B matmuls that
    # keep the PE busy right after warmup.
    cT = [consts.tile([128, L], F16, tag=f"cT{k}", name=f"cT{k}")
          for k in range(NK)]
    # all weights arrive as ONE contiguous [1,769] row = a single 3 KiB
    # descriptor that lands fast on the ACT ring, instead of tiny
    # per-partition descriptors that drain behind the bulk cT/qT traffic.
    # Layout: [cw(256) | cqw(256) | qw(256) | bias(1)].
    wflat = consts.tile([1, 769], F32, name="wflat")
    nc.scalar.dma_start(wflat[:], wflat_d[:, :])
    bias_sb = wflat[0:1, 768:769]
    nc.sync.dma_start(cT[0][:], ct_d[0:128, :])
    nc.sync.dma_start(cT[1][:], ct_d[128:256, :])

    ones_sb = consts.tile([128, MAIN_N], F16, name="ones_sb")
    nc.vector.memset(ones_sb[:], 1.0)

    # ---- PE warmup: release the HAM clock gate during the load window ----
    # The s0acc bank doubles as the warmup target; junk lands in columns the
    # s0 matmuls never touch (and start=True clears has_written anyway).
    s0acc = psum_s0.tile([128, MAIN_N], F32, name="s0acc")
    for w in range(N_WARM):
        nc.tensor.matmul(s0acc[:], ones_sb[:, 0:128], ones_sb[:],
                         start=True, stop=True)

    # ---- on-device weight redistribution via K=1 PE broadcasts -----------
    # out[p, n] = wflat[0, off+p] * 1 gives partition-indexed weight tiles
    # straight from the flat row; tiny matmuls into spare s0acc columns.
    cw16 = consts.tile([128, NK], F16, name="cw16")
    cqw32 = consts.tile([128, NK], F32, name="cqw32")
    ones32 = consts.tile([1, 128], F32, name="ones32")
    nc.vector.memset(ones32[:], 1.0)
    qw_bc = [consts.tile([128, 128], F16, tag=f"qwbc{k}", name=f"qwbc{k}")
             for k in range(NK)]
    for k in range(NK):
        qoff = 512 + 128 * k
        nc.tensor.matmul(s0acc[:, 128 * k:128 * (k + 1)],
                         wflat[0:1, qoff:qoff + 128], ones32[0:1, 0:128],
                         start=True, stop=True)
        nc.vector.tensor_copy(qw_bc[k][:], s0acc[:, 128 * k:128 * (k + 1)])
        nc.tensor.matmul(s0acc[:, 256 + k:257 + k],
                         wflat[0:1, 128 * k:128 * (k + 1)],
                         ones32[0:1, 0:1], start=True, stop=True)
        nc.scalar.activation(cw16[:, k:k + 1], s0acc[:, 256 + k:257 + k],
                             Copy)
        nc.tensor.matmul(s0acc[:, 258 + k:259 + k],
                         wflat[0:1, 256 + 128 * k:256 + 128 * (k + 1)],
                         ones32[0:1, 0:1], start=True, stop=True)
        nc.scalar.activation(cqw32[:, k:k + 1], s0acc[:, 258 + k:259 + k],
                             Copy)

    # ---- transposed fp16 operands ----------------------------------------
    qT = [consts.tile([128, L], F16, tag=f"qT{k}", name=f"qT{k}")
          for k in range(NK)]
    bias_row = consts.tile([1, MAIN_N], F16, name="bias_row")
    nc.scalar.activation(bias_row[0:1, :], ones_sb[0:1, :], Copy,
                         scale=bias_sb)

    for k in range(NK):
        ksl = slice(k * 128, (k + 1) * 128)
        nc.scalar.dma_start(qT[k][:], qt_d[ksl, :])
        # qmodT = qT * cq_weight (per-partition scalar after transpose)
        nc.vector.tensor_scalar_mul(qT[k][:], qT[k][:], cqw32[:, k:k + 1])

    # ---- S1B: s1[j] + bias broadcast across all 128 partitions -----------
    # Accumulation order k0 -> bias -> k1 keeps the PE busy on work that only
    # needs cT[0] while cT[1] is still in flight.
    s1b = consts.tile([128, L], F16, name="s1b")
    s1b_ps = [psum_pb.tile([128, BIG_N], F32, tag="pb", name=f"s1b_ps{t}")
              for t in range(2)]
    for jj in range(4):
        jsl = slice((jj % 2) * MAIN_N, (jj % 2 + 1) * MAIN_N)
        nc.tensor.matmul(s1b_ps[jj // 2][:, jsl], qw_bc[0][:],
                         cT[0][:, jj * MAIN_N:(jj + 1) * MAIN_N],
                         start=True, stop=False)
    for jj in range(4):
        jsl = slice((jj % 2) * MAIN_N, (jj % 2 + 1) * MAIN_N)
        nc.tensor.matmul(s1b_ps[jj // 2][:, jsl], ones_sb[0:1, 0:128],
                         bias_row[0:1, :], start=False, stop=False)
    for jj in range(4):
        jsl = slice((jj % 2) * MAIN_N, (jj % 2 + 1) * MAIN_N)
        nc.tensor.matmul(s1b_ps[jj // 2][:, jsl], qw_bc[1][:],
                         cT[1][:, jj * MAIN_N:(jj + 1) * MAIN_N],
                         start=False, stop=True)
    nc.vector.tensor_copy(s1b[:, 0:BIG_N], s1b_ps[0][:])
    nc.scalar.activation(s1b[:, BIG_N:L], s1b_ps[1][:], Copy)

    s0col = consts.tile([128, NI], F32, name="s0col")

    # ---- main loop: 16 row chunks ----------------------------------------
    for i in range(NI):
        isl = slice(i * 128, (i + 1) * 128)
        out_sb = outp.tile([128, L], F16, tag="out", name="out_sb")
        pa0 = psum_pa.tile([128, MAIN_N], F32, tag="pa", name="pa0")
        pa1 = psum_pa.tile([128, MAIN_N], F32, tag="pa", name="pa1")
        pb = psum_pb.tile([128, BIG_N], F32, tag="pb", name="pb")
        halves = [pa0[:], pa1[:], pb[:, 0:MAIN_N], pb[:, MAIN_N:BIG_N]]
        # pa gets BOTH k-passes first so ScalarE can evacuate it mid-chunk;
        # pb's first touch moves later, giving the previous chunk's
        # evacuations slack before its PSUM slot is reused. The extra two
        # LDWEIGHTS hide under the matmul stream (dual weight buffers).
        # s0's N=1 matmuls lead each k-group (same stationary, no extra LDW).
        for k in range(NK):
            nc.tensor.matmul(s0acc[:, i:i + 1],
                             cT[k][:, isl], cw16[:, k:k + 1],
                             start=(k == 0), stop=(k == NK - 1))
            for jj in range(2):
                nc.tensor.matmul(halves[jj], cT[k][:, isl],
                                 qT[k][:, jj * MAIN_N:(jj + 1) * MAIN_N],
                                 start=(k == 0), stop=(k == NK - 1))
        for k in range(NK):
            for jj in range(2, 4):
                nc.tensor.matmul(halves[jj], cT[k][:, isl],
                                 qT[k][:, jj * MAIN_N:(jj + 1) * MAIN_N],
                                 start=(k == 0), stop=(k == NK - 1))
        # bounce s0 column through SBUF on ScalarE (s0acc stops mid-chunk,
        # so this lands before the next chunk's s0 matmul reuses the bank)
        nc.scalar.activation(s0col[:, i:i + 1], s0acc[:, i:i + 1], Copy)
        tmp = tmpp.tile([128, BIG_N], F16, tag="tmp", name="tmp")
        nc.scalar.activation(tmp[:, 0:MAIN_N], pa0[:], Ident,
                             bias=s0col[:, i:i + 1])
        nc.scalar.activation(tmp[:, MAIN_N:BIG_N], pa1[:], Ident,
                             bias=s0col[:, i:i + 1])
        # pb: fused three-term evacuation on DVE frees the bank promptly
        nc.vector.scalar_tensor_tensor(out_sb[:, BIG_N:L], pb[:],
                                       s0col[:, i:i + 1],
                                       s1b[:, BIG_N:L], ADD, ADD)
        # pa partner adds S1B with plain tensor_tensor (2x fp16 uop)
        if i in GPS_CHUNKS:
            nc.gpsimd.tensor_tensor(out_sb[:, 0:BIG_N], tmp[:],
                                    s1b[:, 0:BIG_N], ADD)
        else:
            nc.vector.tensor_tensor(out_sb[:, 0:BIG_N], tmp[:],
                                    s1b[:, 0:BIG_N], ADD)
        # output DMA in halves: each leaves as soon as its engine finishes
        nc.sync.dma_start(s_d[isl, BIG_N:L], out_sb[:, BIG_N:L])
        nc.sync.dma_start(s_d[isl, 0:BIG_N], out_sb[:, 0:BIG_N])


def build_nc():
    nc = bacc.Bacc("TRN2", target_bir_lowering=False, debug=False)
    aps = {
        "ct": nc.dram_tensor("ct", [D, L], F16, kind="ExternalInput").ap(),
        "qt": nc.dram_tensor("qt", [D, L], F16, kind="ExternalInput").ap(),
        "wflat": nc.dram_tensor("wflat", [1, 769], F32,
                                kind="ExternalInput").ap(),
        "s": nc.dram_tensor("s", [L, L], F16, kind="ExternalOutput").ap(),
    }
    with tile.TileContext(nc) as tc:
        with ExitStack() as ctx:
            build_body(ctx, tc, aps)
    nc.compile()
    return nc


def get_nc():
    global _NC_CACHE
    if _NC_CACHE is None:
        _NC_CACHE = build_nc()
    return _NC_CACHE


def kernel(c, q, c_weight, q_weight, cq_weight, bias):
    global LAST_RESULTS
    nc = get_nc()
    c = np.asarray(c, dtype=np.float32)
    q = np.asarray(q, dtype=np.float32)
    cw = np.asarray(c_weight, dtype=np.float32).reshape(D)
    qw = np.asarray(q_weight, dtype=np.float32).reshape(D)
    cqw = np.asarray(cq_weight, dtype=np.float32).reshape(D)
    bias = np.asarray(bias, dtype=np.float32).reshape(1)

    # flat weight row: [cw(256) | cqw(256) | qw(256) | bias(1)]
    wflat = np.concatenate([cw, cqw, qw, bias]).reshape(1, 769).astype(
        np.float32)

    in_maps = [
        {
            "ct": np.ascontiguousarray(c[b].T).astype(np.float16),
            "qt": np.ascontiguousarray(q[b].T).astype(np.float16),
            "wflat": wflat,
        }
        for b in range(B)
    ]
    res = run_bass_kernel_spmd(nc, in_maps, core_ids=list(range(B)), trace=TRACE)
    LAST_RESULTS = res
    return np.stack([res.results[b]["s"].astype(np.float32) for b in range(B)],
                    axis=0)


# revision 43
# speedup vs baseline: 1.0831x; 1.0831x over previous
"""BiAttention similarity kernel for Trainium2, 8-core data-parallel over batch.

Computes, per batch b:
    s0 = c @ c_weight                  # [L, 1]
    s1 = (c @ q_weight)^T              # [1, L]
    s2 = (c * cq_weight) @ q^T         # [L, L]
    s  = s0 + s1 + s2 + bias           # [L, L]

Shapes (hardcoded): B=8, L=2048, D=256, fp32 in/out (device math fp16/fp32,
device output fp16, upcast to fp32 on host).

Distribution: data-parallel over batch, one batch per core. Host hands each
core its shard d-major (transposed) fp16 plus pre-packed per-partition weight
tiles; device output is fp16 (halves the dominant HBM write).

Device plan per core:
  - warmup matmuls on constant data during the input-load window so the PE's
    HAM clock gate reaches 2.4 GHz before real work arrives
  - S1B [128, L]: s1[j]+bias broadcast across partitions, computed on the PE
    as qw_bcast^T @ cT (+ ones^T @ bias_row), evacuated to SBUF fp16
  - s0 column accumulated in one PSUM bank via N=1 matmuls that reuse the
    main loop's stationary cT chunks, bounced [128,1] per chunk to SBUF
  - main GEMM: 16 row chunks; PSUM tiles are [128,1024] fp32 (two banks) so
    each evacuation instruction covers two banks' worth of output
  - evacuation fuses the rank-1 adds, split across three engines:
      a tiles: DVE scalar_tensor_tensor  out = (psum + s0col) + S1B
      b tiles: ScalarE activation (psum + s0col -> fp16 tmp), then
               DVE tensor_tensor (2x fp16 mode) or GpSimd adds S1B
  - output: one 512 KiB DMA per row chunk on the Sync HWDGE ring
"""

import numpy as np
from contextlib import ExitStack

import concourse.bass as bass
import concourse.tile as tile
from concourse import bacc, mybir
from concourse.bass_utils import run_bass_kernel_spmd

F32 = mybir.dt.float32
F16 = mybir.dt.float16

B = 8
L = 2048
D = 256
NK = D // 128          # 2 contraction chunks of 128
NI = L // 128          # 16 row chunks
MAIN_N = 512           # one matmul output <= one PSUM bank
BIG_N = 1024           # evacuation tile: two PSUM banks
N_WARM = 14            # PE warmup matmuls during input load
GPS_CHUNKS = {0, 1, 2, 4, 5, 6, 8, 9, 10, 12, 13}  # b-tile partner GpSimd

TRACE = False
LAST_RESULTS = None

_NC_CACHE = None


def build_body(ctx: ExitStack, tc: tile.TileContext, aps: dict):
    nc = tc.nc
    ct_d, qt_d, w16_d, w32_d, s_d = (
        aps["ct"], aps["qt"], aps["w16"], aps["w32"], aps["s"],
    )
    Copy = mybir.ActivationFunctionType.Copy
    Ident = mybir.ActivationFunctionType.Identity
    ADD = mybir.AluOpType.add

    consts = ctx.enter_context(tc.tile_pool(name="consts", bufs=1))
    # PSUM budget (8 banks): pa ring 3 x [128,512] = 3 banks, pb ring
    # 2 x [128,1024] = 4 banks (2-chunk depth on the tight edge), s0acc 1
    psum_pa = ctx.enter_context(tc.tile_pool(name="psum_pa", bufs=3,
                                             space="PSUM"))
    psum_pb = ctx.enter_context(tc.tile_pool(name="psum_pb", bufs=2,
                                             space="PSUM"))
    psum_s0 = ctx.enter_context(tc.tile_pool(name="psum_s0", bufs=1,
                                             space="PSUM"))
    outp = ctx.enter_context(tc.tile_pool(name="outp", bufs=4))
    tmpp = ctx.enter_context(tc.tile_pool(name="tmpp", bufs=4))

    # ---- packed constants: one fast HWDGE load each -----------------------
    # w16[p] = [cw[p], cw[128+p]] fp16; w32[p] = [cqw k0, cqw k1, qw k0,
    # qw k1, bias] fp32 (bias only meaningful on partition 0).
    # cT[0] is issued first on the sync ring: it gates the S1B matmuls that
    # keep the PE busy right after warmup.
    cT = [consts.tile([128, L], F16, tag=f"cT{k}", name=f"cT{k}")
          for k in range(NK)]
    # weight packs lead the ACT HWDGE ring: their small-descriptor drain
    # only delays qT (not needed until the first main chunk), while cT has
    # the SP ring to itself
    w16 = consts.tile([128, NK], F16, name="w16")
    nc.scalar.dma_start(w16[:], w16_d[:, :])
    w32 = consts.tile([128, 5], F32, name="w32")
    nc.scalar.dma_start(w32[:], w32_d[:, :])
    nc.sync.dma_start(cT[0][:], ct_d[0:128, :])
    nc.sync.dma_start(cT[1][:], ct_d[128:256, :])
    cw16 = w16
    cqw32 = w32[:, 0:NK]
    qw32 = w32[:, NK:2 * NK]
    bias_sb = w32[0:1, 4:5]

    ones_sb = consts.tile([128, MAIN_N], F16, name="ones_sb")
    nc.vector.memset(ones_sb[:], 1.0)

    # ---- PE warmup: release the HAM clock gate during the load window ----
    # The s0acc bank doubles as the warmup target; junk lands in columns the
    # s0 matmuls never touch (and start=True clears has_written anyway).
    s0acc = psum_s0.tile([128, MAIN_N], F32, name="s0acc")
    for w in range(N_WARM):
        nc.tensor.matmul(s0acc[:], ones_sb[:, 0:128], ones_sb[:],
                         start=True, stop=True)

    # ---- transposed fp16 operands ----------------------------------------
    qT = [consts.tile([128, L], F16, tag=f"qT{k}", name=f"qT{k}")
          for k in range(NK)]
    # qw_bc / bias_row first in the DVE/ACT FIFOs: they only need w32+ones,
    # so S1B's k=0 matmuls can start as soon as cT[0] lands.
    qw_bc = [consts.tile([128, 128], F16, tag=f"qwbc{k}", name=f"qwbc{k}")
             for k in range(NK)]
    for k in range(NK):
        nc.vector.tensor_scalar_mul(qw_bc[k][:], ones_sb[:, 0:128],
                                    qw32[:, k:k + 1])
    bias_row = consts.tile([1, MAIN_N], F16, name="bias_row")
    nc.scalar.activation(bias_row[0:1, :], ones_sb[0:1, :], Copy,
                         scale=bias_sb)

    for k in range(NK):
        ksl = slice(k * 128, (k + 1) * 128)
        nc.scalar.dma_start(qT[k][:], qt_d[ksl, :])
        # qmodT = qT * cq_weight (per-partition scalar after transpose)
        nc.vector.tensor_scalar_mul(qT[k][:], qT[k][:], cqw32[:, k:k + 1])

    # ---- S1B: s1[j] + bias broadcast across all 128 partitions -----------
    # Accumulation order k0 -> bias -> k1 keeps the PE busy on work that only
    # needs cT[0] while cT[1] is still in flight.
    s1b = consts.tile([128, L], F16, name="s1b")
    s1b_ps = [psum_pb.tile([128, BIG_N], F32, tag="pb", name=f"s1b_ps{t}")
              for t in range(2)]
    for jj in range(4):
        jsl = slice((jj % 2) * MAIN_N, (jj % 2 + 1) * MAIN_N)
        nc.tensor.matmul(s1b_ps[jj // 2][:, jsl], qw_bc[0][:],
                         cT[0][:, jj * MAIN_N:(jj + 1) * MAIN_N],
                         start=True, stop=False)
    for jj in range(4):
        jsl = slice((jj % 2) * MAIN_N, (jj % 2 + 1) * MAIN_N)
        nc.tensor.matmul(s1b_ps[jj // 2][:, jsl], ones_sb[0:1, 0:128],
                         bias_row[0:1, :], start=False, stop=False)
    for jj in range(4):
        jsl = slice((jj % 2) * MAIN_N, (jj % 2 + 1) * MAIN_N)
        nc.tensor.matmul(s1b_ps[jj // 2][:, jsl], qw_bc[1][:],
                         cT[1][:, jj * MAIN_N:(jj + 1) * MAIN_N],
                         start=False, stop=True)
    nc.vector.tensor_copy(s1b[:, 0:BIG_N], s1b_ps[0][:])
    nc.scalar.activation(s1b[:, BIG_N:L], s1b_ps[1][:], Copy)

    s0col = consts.tile([128, NI], F32, name="s0col")

    # ---- main loop: 16 row chunks ----------------------------------------
    for i in range(NI):
        isl = slice(i * 128, (i + 1) * 128)
        out_sb = outp.tile([128, L], F16, tag="out", name="out_sb")
        pa0 = psum_pa.tile([128, MAIN_N], F32, tag="pa", name="pa0")
        pa1 = psum_pa.tile([128, MAIN_N], F32, tag="pa", name="pa1")
        pb = psum_pb.tile([128, BIG_N], F32, tag="pb", name="pb")
        halves = [pa0[:], pa1[:], pb[:, 0:MAIN_N], pb[:, MAIN_N:BIG_N]]
        # pa gets BOTH k-passes first so ScalarE can evacuate it mid-chunk;
        # pb's first touch moves later, giving the previous chunk's
        # evacuations slack before its PSUM slot is reused. The extra two
        # LDWEIGHTS hide under the matmul stream (dual weight buffers).
        # s0's N=1 matmuls lead each k-group (same stationary, no extra LDW).
        for k in range(NK):
            nc.tensor.matmul(s0acc[:, i:i + 1],
                             cT[k][:, isl], cw16[:, k:k + 1],
                             start=(k == 0), stop=(k == NK - 1))
            for jj in range(2):
                nc.tensor.matmul(halves[jj], cT[k][:, isl],
                                 qT[k][:, jj * MAIN_N:(jj + 1) * MAIN_N],
                                 start=(k == 0), stop=(k == NK - 1))
        for k in range(NK):
            for jj in range(2, 4):
                nc.tensor.matmul(halves[jj], cT[k][:, isl],
                                 qT[k][:, jj * MAIN_N:(jj + 1) * MAIN_N],
                                 start=(k == 0), stop=(k == NK - 1))
        # bounce s0 column through SBUF on ScalarE (s0acc stops mid-chunk,
        # so this lands before the next chunk's s0 matmul reuses the bank)
        nc.scalar.activation(s0col[:, i:i + 1], s0acc[:, i:i + 1], Copy)
        tmp = tmpp.tile([128, BIG_N], F16, tag="tmp", name="tmp")
        nc.scalar.activation(tmp[:, 0:MAIN_N], pa0[:], Ident,
                             bias=s0col[:, i:i + 1])
        nc.scalar.activation(tmp[:, MAIN_N:BIG_N], pa1[:], Ident,
                             bias=s0col[:, i:i + 1])
        # pb: fused three-term evacuation on DVE frees the bank promptly
        nc.vector.scalar_tensor_tensor(out_sb[:, BIG_N:L], pb[:],
                                       s0col[:, i:i + 1],
                                       s1b[:, BIG_N:L], ADD, ADD)
        # pa partner adds S1B with plain tensor_tensor (2x fp16 uop)
        if i in GPS_CHUNKS:
            nc.gpsimd.tensor_tensor(out_sb[:, 0:BIG_N], tmp[:],
                                    s1b[:, 0:BIG_N], ADD)
        else:
            nc.vector.tensor_tensor(out_sb[:, 0:BIG_N], tmp[:],
                                    s1b[:, 0:BIG_N], ADD)
        # output DMA in halves: each leaves as soon as its engine finishes
        nc.sync.dma_start(s_d[isl, BIG_N:L], out_sb[:, BIG_N:L])
        nc.sync.dma_start(s_d[isl, 0:BIG_N], out_sb[:, 0:BIG_N])


def build_nc():
    nc = bacc.Bacc("TRN2", target_bir_lowering=False, debug=False)
    aps = {
        "ct": nc.dram_tensor("ct", [D, L], F16, kind="ExternalInput").ap(),
        "qt": nc.dram_tensor("qt", [D, L], F16, kind="ExternalInput").ap(),
        "w16": nc.dram_tensor("w16", [128, NK], F16, kind="ExternalInput").ap(),
        "w32": nc.dram_tensor("w32", [128, 5], F32, kind="ExternalInput").ap(),
        "s": nc.dram_tensor("s", [L, L], F16, kind="ExternalOutput").ap(),
    }
    with tile.TileContext(nc) as tc:
        with ExitStack() as ctx:
            build_body(ctx, tc, aps)
    nc.compile()
    return nc


def get_nc():
    global _NC_CACHE
    if _NC_CACHE is None:
        _NC_CACHE = build_nc()
    return _NC_CACHE


def kernel(c, q, c_weight, q_weight, cq_weight, bias):
    global LAST_RESULTS
    nc = get_nc()
    c = np.asarray(c, dtype=np.float32)
    q = np.asarray(q, dtype=np.float32)
    cw = np.asarray(c_weight, dtype=np.float32).reshape(D)
    qw = np.asarray(q_weight, dtype=np.float32).reshape(D)
    cqw = np.asarray(cq_weight, dtype=np.float32).reshape(D)
    bias = np.asarray(bias, dtype=np.float32).reshape(1)

    # packed per-partition weights: row p of w16 = [cw[p], cw[128+p]] fp16;
    # row p of w32 = [cqw[p], cqw[128+p], qw[p], qw[128+p], bias]
    w16 = np.ascontiguousarray(cw.reshape(NK, 128).T).astype(np.float16)
    w32 = np.empty((128, 5), dtype=np.float32)
    w32[:, 0:NK] = cqw.reshape(NK, 128).T
    w32[:, NK:2 * NK] = qw.reshape(NK, 128).T
    w32[:, 4] = bias[0]

    in_maps = [
        {
            "ct": np.ascontiguousarray(c[b].T).astype(np.float16),
            "qt": np.ascontiguousarray(q[b].T).astype(np.float16),
            "w16": w16,
            "w32": w32,
        }
        for b in range(B)
    ]
    res = run_bass_kernel_spmd(nc, in_maps, core_ids=list(range(B)), trace=TRACE)
    LAST_RESULTS = res
    return np.stack([res.results[b]["s"].astype(np.float32) for b in range(B)],
                    axis=0)
